# revision 17
# baseline (speedup 1.0000x reference)
"""Trainium2 Bass kernel for nn_Attention (dense transformer attention).

Full module: qkv projection -> per-head softmax(q k^T / sqrt(d)) -> attn @ v
-> output projection (+bias). Returns (out, attn) like the reference.

Distribution: sequence-parallel over 8 NeuronCores. Each core owns a block of
512 query rows and computes, fully on-device:
  - k^T and v for ALL 4096 tokens (replicated compute; beats an all-gather,
    whose fixed cost measures ~200us on this stack),
  - q^T for its own 512 rows,
  - per head pair: dots^T -> exp -> E^T (bf16) feeding O^T = (E @ v)^T on the
    TensorEngine (two heads packed into PE row groups / PSUM col groups),
    then row-major dots -> exp (with row-sum accumulation) -> normalized
    attention rows written straight to HBM,
  - O^T normalized via PE-transposed reciprocal row sums, then the output
    projection for its own rows (bias folded in as a K=1 matmul).

Pair 0's phase A AND phase B are woven into the tail of stage 1 so the
ScalarEngine and DMA engines are busy while the TensorEngine grinds through
the qkv projections (keeps the PE HAM clock-gate warm, overlaps the first
16MB of attention writes).

Host side only reshapes/casts inputs (x^T, weights to bf16) and concatenates
the per-core outputs. No FLOPs on the host.
"""

import sys

sys.path.insert(0, "/opt/trn_rl_repo")

import numpy as np
import ml_dtypes

import concourse.bass as bass
import concourse.mybir as mybir
import concourse.tile as tile
from concourse import bacc
from concourse.bass_utils import run_bass_kernel_spmd
from concourse.masks import make_identity

F32 = mybir.dt.float32
BF16 = mybir.dt.bfloat16
EXP = mybir.ActivationFunctionType.Exp
AX_X = mybir.AxisListType.X

N = 4096          # sequence length
D = 512           # model dim
H = 8             # heads
DH = 64           # head dim
NC = 8            # cores
IB = N // NC      # query rows per core = 512
P = 128
SCALE = DH ** -0.5

KC = D // P        # 4 contraction chunks for the qkv projections
NJT = N // P       # 32 key tiles
NIT = IB // P      # 4 query row-tiles per core
HP = H // 2        # head pairs
JC2 = N // (2 * D) # 4 row-major 1024-wide chunks

_nc_cache = {}


def build_nc():
    """Build the single-core Bass program (same program runs SPMD on 8 cores)."""
    nc = bacc.Bacc("TRN2", target_bir_lowering=False, debug=False,
                   enable_asserts=False)

    xT = nc.declare_dram_parameter("xT", [D, N], BF16, isOutput=False)
    xTq = nc.declare_dram_parameter("xTq", [D, IB], BF16, isOutput=False)
    wqkv = nc.declare_dram_parameter("w_qkv", [D, 3 * D], BF16, isOutput=False)
    wout = nc.declare_dram_parameter("w_out", [D, D], BF16, isOutput=False)
    bout = nc.declare_dram_parameter("b_out", [1, D], BF16, isOutput=False)
    attn_o = nc.declare_dram_parameter("attn", [H, IB, N], F32, isOutput=True)
    out_o = nc.declare_dram_parameter("out", [IB, D], F32, isOutput=True)

    with tile.TileContext(nc) as tc:
        with (
            tc.tile_pool(name="consts", bufs=1) as consts,
            tc.tile_pool(name="kt2", bufs=1) as kt2_pool,
            tc.tile_pool(name="vsb", bufs=1) as v_pool,
            tc.tile_pool(name="qt2", bufs=1) as qt2_pool,
            tc.tile_pool(name="otp", bufs=1) as ot_pool,
            tc.tile_pool(name="srows", bufs=1) as srow_pool,
            tc.tile_pool(name="et", bufs=3) as et_pool,
            tc.tile_pool(name="erow", bufs=4) as erow_pool,
            tc.tile_pool(name="sacc", bufs=4) as sacc_pool,
            tc.tile_pool(name="bcast", bufs=1) as bc_pool,
            tc.tile_pool(name="osb", bufs=1) as out_pool,
            tc.tile_pool(name="psAB", bufs=3, space="PSUM") as psAB,
            tc.tile_pool(name="psO", bufs=2, space="PSUM") as psO,
        ):
            wout_sb = [consts.tile([P, D], BF16, name=f"wout{t}", tag=f"wout{t}")
                       for t in range(KC)]
            bout_sb = consts.tile([1, D], BF16, name="bout", tag="bout")
            ones_bf = consts.tile([1, P], BF16, name="ones_bf", tag="ones_bf")
            ones_f32 = consts.tile([1, DH], F32, name="ones_f32", tag="ones_f32")
            ident = consts.tile([P, P], F32, name="ident", tag="ident")
            for t in range(KC):
                nc.sync.dma_start(wout_sb[t][:], wout[t * P:(t + 1) * P, :])
            nc.sync.dma_start(bout_sb[:], bout[:])
            nc.vector.memset(ones_bf[:], 1.0)
            nc.vector.memset(ones_f32[:], 1.0)
            make_identity(nc, ident[:])

            kt2 = [kt2_pool.tile([P, N], BF16, name=f"kt2_{hp}", tag=f"kt2_{hp}")
                   for hp in range(HP)]
            v_sb = [v_pool.tile([P, D], BF16, name=f"v{jt}", tag=f"v{jt}")
                    for jt in range(NJT)]
            qt2 = [qt2_pool.tile([P, IB], BF16, name=f"qt2_{hp}", tag=f"qt2_{hp}")
                   for hp in range(HP)]
            ot_norm = [ot_pool.tile([P, IB], BF16, name=f"otn{t}", tag=f"otn{t}")
                       for t in range(HP)]
            rs_h = [srow_pool.tile([P, NIT], F32, name=f"rs_{h}", tag=f"rs_{h}")
                    for h in range(H)]

            # ================= stage 1 emitters =================
            with tc.tile_pool(name="xin", bufs=1) as xin_pool:
                xT_sb = [xin_pool.tile([P, N], BF16, name=f"xT{kc}", tag=f"xT{kc}")
                         for kc in range(KC)]
                xTq_sb = [xin_pool.tile([P, IB], BF16, name=f"xTq{kc}", tag=f"xTq{kc}")
                          for kc in range(KC)]
                wqkv_sb = [xin_pool.tile([P, 3 * D], BF16, name=f"wqkv{kc}", tag=f"wqkv{kc}")
                           for kc in range(KC)]
                for kc in range(KC):
                    nc.sync.dma_start(xT_sb[kc][:], xT[kc * P:(kc + 1) * P, :])
                    nc.sync.dma_start(xTq_sb[kc][:], xTq[kc * P:(kc + 1) * P, :])
                    nc.sync.dma_start(wqkv_sb[kc][:], wqkv[kc * P:(kc + 1) * P, :])

                def emit_kt_chunk(ft, jc2):
                    ps = psAB.tile([P, 2 * D], F32, name="psAB", tag="psAB")
                    for half in range(2):
                        jc = 2 * jc2 + half
                        for kc in range(KC):
                            nc.tensor.matmul(
                                ps[:, half * D:(half + 1) * D],
                                wqkv_sb[kc][:, D + ft * P: D + (ft + 1) * P],
                                xT_sb[kc][:, jc * D:(jc + 1) * D],
                                start=(kc == 0), stop=(kc == KC - 1))
                    nc.vector.tensor_copy(
                        kt2[ft][:, jc2 * 2 * D:(jc2 + 1) * 2 * D], ps[:])

                def emit_qt(ft):
                    ps = psAB.tile([P, 2 * D], F32, name="psAB", tag="psAB")
                    for kc in range(KC):
                        nc.tensor.matmul(
                            ps[:, 0:IB],
                            wqkv_sb[kc][:, ft * P:(ft + 1) * P],
                            xTq_sb[kc][:],
                            start=(kc == 0), stop=(kc == KC - 1))
                    nc.vector.tensor_copy(qt2[ft][:], ps[:, 0:IB])

                def emit_v(jt):
                    ps = psO.tile([P, D], F32, name="psO", tag="psO")
                    for kc in range(KC):
                        nc.tensor.matmul(
                            ps[:], xT_sb[kc][:, jt * P:(jt + 1) * P],
                            wqkv_sb[kc][:, 2 * D:3 * D],
                            start=(kc == 0), stop=(kc == KC - 1))
                    nc.vector.tensor_copy(v_sb[jt][:], ps[:])

                # ================= stage 2 emitters =================
                def emit_a_step(hp, jt, ops_pair):
                    h0, h1 = 2 * hp, 2 * hp + 1
                    dps = psAB.tile([P, 2 * IB], F32, name="psAB", tag="psAB")
                    nc.tensor.matmul(
                        dps[:, 0:IB], kt2[hp][0:DH, jt * P:(jt + 1) * P],
                        qt2[hp][0:DH, :], start=True, stop=True)
                    nc.tensor.matmul(
                        dps[:, IB:2 * IB], kt2[hp][DH:P, jt * P:(jt + 1) * P],
                        qt2[hp][DH:P, :], start=True, stop=True)
                    et = et_pool.tile([P, 2 * IB], BF16, name="et", tag="et")
                    nc.scalar.activation(et[:], dps[:], EXP, scale=SCALE)
                    nc.tensor.matmul(ops_pair[0:DH, :],
                                     v_sb[jt][:, h0 * DH:(h0 + 1) * DH],
                                     et[:, 0:IB],
                                     start=(jt == 0), stop=(jt == NJT - 1))
                    nc.tensor.matmul(ops_pair[DH:P, :],
                                     v_sb[jt][:, h1 * DH:(h1 + 1) * DH],
                                     et[:, IB:2 * IB],
                                     start=(jt == 0), stop=(jt == NJT - 1),
                                     tile_position=(0, DH))

                b_state = {}

                def open_b_state(hp):
                    for h in (2 * hp, 2 * hp + 1):
                        for it in range(NIT):
                            b_state[(h, it)] = {
                                "erow": erow_pool.tile([P, N], F32, name="erow", tag="erow"),
                                "sacc": sacc_pool.tile([P, JC2], F32, name="sacc", tag="sacc"),
                            }

                def emit_b_unit(hp, it, jc2):
                    h0, h1 = 2 * hp, 2 * hp + 1
                    rps0 = psAB.tile([P, 2 * D], F32, name="psAB", tag="psAB")
                    rps1 = psAB.tile([P, 2 * D], F32, name="psAB", tag="psAB")
                    for half in range(2):
                        jcs = (2 * jc2 + half) * D
                        for rps, pb in ((rps0, 0), (rps1, DH)):
                            nc.tensor.matmul(
                                rps[:, half * D:(half + 1) * D],
                                qt2[hp][pb:pb + DH, it * P:(it + 1) * P],
                                kt2[hp][pb:pb + DH, jcs:jcs + D],
                                start=True, stop=True)
                    for rps, h in ((rps0, h0), (rps1, h1)):
                        st = b_state[(h, it)]
                        nc.scalar.activation(
                            st["erow"][:, jc2 * 2 * D:(jc2 + 1) * 2 * D], rps[:],
                            EXP, scale=SCALE,
                            accum_out=st["sacc"][:, jc2:jc2 + 1])
                        if jc2 == JC2 - 1:
                            stot = sacc_pool.tile([P, 1], F32, name="stot", tag="stot")
                            nc.vector.reduce_sum(stot[:], st["sacc"][:], axis=AX_X)
                            nc.vector.reciprocal(rs_h[h][:, it:it + 1], stot[:])
                            nc.vector.tensor_scalar_mul(st["erow"][:], st["erow"][:],
                                                        rs_h[h][:, it:it + 1])
                            nc.sync.dma_start(
                                attn_o[h, it * P:(it + 1) * P, :], st["erow"][:])

                def normalize_pair(hp, ops_pair):
                    h0, h1 = 2 * hp, 2 * hp + 1
                    bps = psAB.tile([P, 2 * D], F32, name="psAB", tag="psAB")
                    for idx, h in enumerate((h0, h1)):
                        rs_row = bc_pool.tile([1, IB], F32, name="rs_row", tag="rs_row")
                        for it in range(NIT):
                            tps = psO.tile([P, D], F32, name="psO", tag="psO")
                            nc.tensor.transpose(tps[0:1, 0:P], rs_h[h][:, it:it + 1],
                                                ident[:])
                            nc.vector.tensor_copy(rs_row[:, it * P:(it + 1) * P],
                                                  tps[0:1, 0:P])
                        nc.tensor.matmul(bps[idx * DH:(idx + 1) * DH, 0:IB],
                                         ones_f32[:], rs_row[:],
                                         start=True, stop=True,
                                         tile_position=(0, idx * DH))
                    bsb = bc_pool.tile([P, IB], F32, name="bsb", tag="bsb")
                    nc.vector.tensor_copy(bsb[:], bps[:, 0:IB])
                    nc.vector.tensor_mul(ot_norm[hp][:], ops_pair[:], bsb[:])

                # ---------- stage 1 core + woven pair 0 ----------
                for jt in range(NJT):
                    emit_v(jt)
                for jc2 in range(JC2):
                    emit_kt_chunk(0, jc2)
                emit_qt(0)

                # weave remaining stage-1 work + all of pair 0 (A and B);
                # pair 1's k/q come first so its B phase can also run early
                stage1_rest = [("kt", 1, jc2) for jc2 in range(JC2)]
                stage1_rest += [("qt", 1, 0)]
                stage1_rest += [("kt", ft, jc2) for ft in range(2, KC)
                                for jc2 in range(JC2)]
                stage1_rest += [("qt", ft, 0) for ft in range(2, KC)]
                s1_iter = iter(stage1_rest)
                open_b_state(0)
                b0_units = [(it, jc2) for it in range(NIT) for jc2 in range(JC2)]
                b0_iter = iter(b0_units)
                ops_pair0 = psO.tile([P, IB], F32, name="psO", tag="psO")
                for jt in range(NJT):
                    emit_a_step(0, jt, ops_pair0)
                    if jt % 2 == 0:
                        nxt = next(s1_iter, None)
                        if nxt is not None:
                            kind, ft, jc2 = nxt
                            if kind == "kt":
                                emit_kt_chunk(ft, jc2)
                            else:
                                emit_qt(ft)
                    elif jt >= 8:
                        u = next(b0_iter, None)
                        if u is not None:
                            emit_b_unit(0, *u)
                for nxt in s1_iter:
                    kind, ft, jc2 = nxt
                    if kind == "kt":
                        emit_kt_chunk(ft, jc2)
                    else:
                        emit_qt(ft)
                for u in b0_iter:
                    emit_b_unit(0, *u)
                normalize_pair(0, ops_pair0)

                # pair 1's B runs in the region-1 tail (ACT/DMA slack there)
                open_b_state(1)
                for it in range(NIT):
                    for jc2 in range(JC2):
                        emit_b_unit(1, it, jc2)
                ops_pair1 = psO.tile([P, IB], F32, name="psO", tag="psO")
                for jt in range(NJT):
                    emit_a_step(1, jt, ops_pair1)
                normalize_pair(1, ops_pair1)

                # ---------- remaining pairs ----------
                for hp in range(2, HP):
                    ops_pair = psO.tile([P, IB], F32, name="psO", tag="psO")
                    for jt in range(NJT):
                        emit_a_step(hp, jt, ops_pair)
                    open_b_state(hp)
                    for it in range(NIT):
                        for jc2 in range(JC2):
                            emit_b_unit(hp, it, jc2)
                    normalize_pair(hp, ops_pair)

                # ---------- output projection ----------
                for it in range(NIT):
                    pps = psAB.tile([P, 2 * D], F32, name="psAB", tag="psAB")
                    for t in range(HP):
                        nc.tensor.matmul(pps[:, 0:D],
                                         ot_norm[t][:, it * P:(it + 1) * P],
                                         wout_sb[t][:], start=(t == 0), stop=False)
                    nc.tensor.matmul(pps[:, 0:D], ones_bf[:], bout_sb[:],
                                     start=False, stop=True)
                    osb = out_pool.tile([P, D], F32, name="osb", tag="osb")
                    nc.vector.tensor_copy(osb[:], pps[:, 0:D])
                    nc.sync.dma_start(out_o[it * P:(it + 1) * P, :], osb[:])

    nc.compile()
    return nc


def _get_nc():
    if "nc" not in _nc_cache:
        _nc_cache["nc"] = build_nc()
    return _nc_cache["nc"]


def _prep_in_maps(x, W_qkv, W_out, b_out):
    bf = ml_dtypes.bfloat16
    xT = np.ascontiguousarray(np.asarray(x, np.float32)[0].T).astype(bf)  # [512, 4096]
    wqkv = np.ascontiguousarray(np.asarray(W_qkv, np.float32)).astype(bf)
    wout = np.ascontiguousarray(np.asarray(W_out, np.float32)).astype(bf)
    bo = np.asarray(b_out, np.float32).reshape(1, D).astype(bf)
    in_maps = []
    for c in range(NC):
        xTq = np.ascontiguousarray(xT[:, c * IB:(c + 1) * IB])
        in_maps.append({"xT": xT, "xTq": xTq, "w_qkv": wqkv,
                        "w_out": wout, "b_out": bo})
    return in_maps


def run(inputs, trace=False, tmpdir=None):
    nc = _get_nc()
    in_maps = _prep_in_maps(inputs["x"], inputs["W_qkv"], inputs["W_out"],
                            inputs["b_out"])
    res = run_bass_kernel_spmd(nc, in_maps, core_ids=list(range(NC)),
                               trace=trace, tmpdir=tmpdir)
    outs = [res.results[c]["out"] for c in range(NC)]
    attns = [res.results[c]["attn"] for c in range(NC)]
    out_full = np.concatenate(outs, axis=0)[None]                   # [1, 4096, 512]
    attn_full = np.concatenate(attns, axis=1)[None]                 # [1, 8, 4096, 4096]
    return (out_full.astype(np.float32), attn_full.astype(np.float32)), res


def kernel(**inputs):
    (out_full, attn_full), _ = run(inputs, trace=False)
    return out_full, attn_full


# revision 22
# speedup vs baseline: 1.0440x; 1.0440x over previous
"""Trainium2 Bass kernel for nn_Attention (dense transformer attention).

Full module: qkv projection -> per-head softmax(q k^T / sqrt(d)) -> attn @ v
-> output projection (+bias). Returns (out, attn) like the reference.

Distribution: sequence-parallel over 8 NeuronCores. Each core owns a block of
512 query rows and computes, fully on-device:
  - k^T and v for ALL 4096 tokens (replicated compute; beats an all-gather,
    whose fixed cost measures ~200us on this stack),
  - q^T for its own 512 rows,
  - per head pair: dots^T -> exp -> E^T (bf16) feeding O^T = (E @ v)^T on the
    TensorEngine (two heads packed into PE row groups / PSUM col groups),
    then row-major dots -> exp (with row-sum accumulation) -> normalized
    attention rows written straight to HBM,
  - O^T normalized via PE-transposed reciprocal row sums, then the output
    projection for its own rows (bias folded in as a K=1 matmul).

Pair 0's phase A AND phase B are woven into the tail of stage 1 so the
ScalarEngine and DMA engines are busy while the TensorEngine grinds through
the qkv projections (keeps the PE HAM clock-gate warm, overlaps the first
16MB of attention writes).

Host side only reshapes/casts inputs (x^T, weights to bf16) and concatenates
the per-core outputs. No FLOPs on the host.
"""

import sys

sys.path.insert(0, "/opt/trn_rl_repo")

import numpy as np
import ml_dtypes

import concourse.bass as bass
import concourse.mybir as mybir
import concourse.tile as tile
from concourse import bacc
from concourse.bass_utils import run_bass_kernel_spmd
from concourse.masks import make_identity

F32 = mybir.dt.float32
BF16 = mybir.dt.bfloat16
EXP = mybir.ActivationFunctionType.Exp
AX_X = mybir.AxisListType.X

N = 4096          # sequence length
D = 512           # model dim
H = 8             # heads
DH = 64           # head dim
NC = 8            # cores
IB = N // NC      # query rows per core = 512
P = 128
SCALE = DH ** -0.5

KC = D // P        # 4 contraction chunks for the qkv projections
NJT = N // P       # 32 key tiles
NIT = IB // P      # 4 query row-tiles per core
HP = H // 2        # head pairs
JC2 = N // (2 * D) # 4 row-major 1024-wide chunks

_nc_cache = {}


def build_nc():
    """Build the single-core Bass program (same program runs SPMD on 8 cores)."""
    nc = bacc.Bacc("TRN2", target_bir_lowering=False, debug=False,
                   enable_asserts=False)

    xT = nc.declare_dram_parameter("xT", [D, N], BF16, isOutput=False)
    xTq = nc.declare_dram_parameter("xTq", [D, IB], BF16, isOutput=False)
    wqkv = nc.declare_dram_parameter("w_qkv", [D, 3 * D], BF16, isOutput=False)
    wout = nc.declare_dram_parameter("w_out", [D, D], BF16, isOutput=False)
    bout = nc.declare_dram_parameter("b_out", [1, D], BF16, isOutput=False)
    attn_o = nc.declare_dram_parameter("attn", [H, IB, N], F32, isOutput=True)
    out_o = nc.declare_dram_parameter("out", [IB, D], F32, isOutput=True)

    with tile.TileContext(nc) as tc:
        with (
            tc.tile_pool(name="consts", bufs=1) as consts,
            tc.tile_pool(name="kt2", bufs=1) as kt2_pool,
            tc.tile_pool(name="vsb", bufs=1) as v_pool,
            tc.tile_pool(name="qt2", bufs=1) as qt2_pool,
            tc.tile_pool(name="otp", bufs=1) as ot_pool,
            tc.tile_pool(name="srows", bufs=1) as srow_pool,
            tc.tile_pool(name="et", bufs=4) as et_pool,
            tc.tile_pool(name="erow", bufs=8) as erow_pool,
            tc.tile_pool(name="sacc", bufs=4) as sacc_pool,
            tc.tile_pool(name="bcast", bufs=1) as bc_pool,
            tc.tile_pool(name="osb", bufs=1) as out_pool,
            tc.tile_pool(name="psAB", bufs=3, space="PSUM") as psAB,
            tc.tile_pool(name="psO", bufs=2, space="PSUM") as psO,
        ):
            wout_sb = [consts.tile([P, D], BF16, name=f"wout{t}", tag=f"wout{t}")
                       for t in range(KC)]
            bout_sb = consts.tile([1, D], BF16, name="bout", tag="bout")
            ones_bf = consts.tile([1, P], BF16, name="ones_bf", tag="ones_bf")
            ones_f32 = consts.tile([1, DH], F32, name="ones_f32", tag="ones_f32")
            ident = consts.tile([P, P], F32, name="ident", tag="ident")
            for t in range(KC):
                nc.sync.dma_start(wout_sb[t][:], wout[t * P:(t + 1) * P, :])
            nc.sync.dma_start(bout_sb[:], bout[:])
            nc.vector.memset(ones_bf[:], 1.0)
            nc.vector.memset(ones_f32[:], 1.0)
            make_identity(nc, ident[:])

            kt2 = [kt2_pool.tile([P, N], BF16, name=f"kt2_{hp}", tag=f"kt2_{hp}")
                   for hp in range(HP)]
            v_sb = [v_pool.tile([P, D], BF16, name=f"v{jt}", tag=f"v{jt}")
                    for jt in range(NJT)]
            qt2 = [qt2_pool.tile([P, IB], BF16, name=f"qt2_{hp}", tag=f"qt2_{hp}")
                   for hp in range(HP)]
            ot_norm = [ot_pool.tile([P, IB], BF16, name=f"otn{t}", tag=f"otn{t}")
                       for t in range(HP)]
            rs_h = [srow_pool.tile([P, NIT], F32, name=f"rs_{h}", tag=f"rs_{h}")
                    for h in range(H)]

            # ================= stage 1 emitters =================
            with tc.tile_pool(name="xin", bufs=1) as xin_pool:
                xT_sb = [xin_pool.tile([P, N], BF16, name=f"xT{kc}", tag=f"xT{kc}")
                         for kc in range(KC)]
                xTq_sb = [xin_pool.tile([P, IB], BF16, name=f"xTq{kc}", tag=f"xTq{kc}")
                          for kc in range(KC)]
                wqkv_sb = [xin_pool.tile([P, 3 * D], BF16, name=f"wqkv{kc}", tag=f"wqkv{kc}")
                           for kc in range(KC)]
                for kc in range(KC):
                    nc.sync.dma_start(xT_sb[kc][:], xT[kc * P:(kc + 1) * P, :])
                    nc.sync.dma_start(xTq_sb[kc][:], xTq[kc * P:(kc + 1) * P, :])
                    nc.sync.dma_start(wqkv_sb[kc][:], wqkv[kc * P:(kc + 1) * P, :])

                def emit_kt_chunk(ft, jc2):
                    ps = psAB.tile([P, 2 * D], F32, name="psAB", tag="psAB")
                    for half in range(2):
                        jc = 2 * jc2 + half
                        for kc in range(KC):
                            nc.tensor.matmul(
                                ps[:, half * D:(half + 1) * D],
                                wqkv_sb[kc][:, D + ft * P: D + (ft + 1) * P],
                                xT_sb[kc][:, jc * D:(jc + 1) * D],
                                start=(kc == 0), stop=(kc == KC - 1))
                    nc.vector.tensor_copy(
                        kt2[ft][:, jc2 * 2 * D:(jc2 + 1) * 2 * D], ps[:])

                def emit_qt(ft):
                    ps = psAB.tile([P, 2 * D], F32, name="psAB", tag="psAB")
                    for kc in range(KC):
                        nc.tensor.matmul(
                            ps[:, 0:IB],
                            wqkv_sb[kc][:, ft * P:(ft + 1) * P],
                            xTq_sb[kc][:],
                            start=(kc == 0), stop=(kc == KC - 1))
                    nc.vector.tensor_copy(qt2[ft][:], ps[:, 0:IB])

                def emit_v(jt):
                    ps = psO.tile([P, D], F32, name="psO", tag="psO")
                    for kc in range(KC):
                        nc.tensor.matmul(
                            ps[:], xT_sb[kc][:, jt * P:(jt + 1) * P],
                            wqkv_sb[kc][:, 2 * D:3 * D],
                            start=(kc == 0), stop=(kc == KC - 1))
                    nc.vector.tensor_copy(v_sb[jt][:], ps[:])

                # ================= stage 2 emitters =================
                def emit_a_step(hp, jt, ops_pair):
                    h0, h1 = 2 * hp, 2 * hp + 1
                    dps = psAB.tile([P, 2 * IB], F32, name="psAB", tag="psAB")
                    nc.tensor.matmul(
                        dps[:, 0:IB], kt2[hp][0:DH, jt * P:(jt + 1) * P],
                        qt2[hp][0:DH, :], start=True, stop=True)
                    nc.tensor.matmul(
                        dps[:, IB:2 * IB], kt2[hp][DH:P, jt * P:(jt + 1) * P],
                        qt2[hp][DH:P, :], start=True, stop=True)
                    et = et_pool.tile([P, 2 * IB], BF16, name="et", tag="et")
                    nc.scalar.activation(et[:], dps[:], EXP, scale=SCALE)
                    nc.tensor.matmul(ops_pair[0:DH, :],
                                     v_sb[jt][:, h0 * DH:(h0 + 1) * DH],
                                     et[:, 0:IB],
                                     start=(jt == 0), stop=(jt == NJT - 1))
                    nc.tensor.matmul(ops_pair[DH:P, :],
                                     v_sb[jt][:, h1 * DH:(h1 + 1) * DH],
                                     et[:, IB:2 * IB],
                                     start=(jt == 0), stop=(jt == NJT - 1),
                                     tile_position=(0, DH))

                b_state = {}

                def open_b_state(hp):
                    for h in (2 * hp, 2 * hp + 1):
                        for it in range(NIT):
                            b_state[(h, it)] = {
                                "erow": erow_pool.tile([P, N], BF16, name="erow", tag="erow"),
                                "sacc": sacc_pool.tile([P, JC2], F32, name="sacc", tag="sacc"),
                            }

                def emit_b_unit(hp, it, jc2):
                    h0, h1 = 2 * hp, 2 * hp + 1
                    rps0 = psAB.tile([P, 2 * D], F32, name="psAB", tag="psAB")
                    rps1 = psAB.tile([P, 2 * D], F32, name="psAB", tag="psAB")
                    for half in range(2):
                        jcs = (2 * jc2 + half) * D
                        for rps, pb in ((rps0, 0), (rps1, DH)):
                            nc.tensor.matmul(
                                rps[:, half * D:(half + 1) * D],
                                qt2[hp][pb:pb + DH, it * P:(it + 1) * P],
                                kt2[hp][pb:pb + DH, jcs:jcs + D],
                                start=True, stop=True)
                    for rps, h in ((rps0, h0), (rps1, h1)):
                        st = b_state[(h, it)]
                        nc.scalar.activation(
                            st["erow"][:, jc2 * 2 * D:(jc2 + 1) * 2 * D], rps[:],
                            EXP, scale=SCALE,
                            accum_out=st["sacc"][:, jc2:jc2 + 1])
                        if jc2 == JC2 - 1:
                            stot = sacc_pool.tile([P, 1], F32, name="stot", tag="stot")
                            nc.vector.reduce_sum(stot[:], st["sacc"][:], axis=AX_X)
                            nc.vector.reciprocal(rs_h[h][:, it:it + 1], stot[:])
                            nc.vector.tensor_scalar_mul(st["erow"][:], st["erow"][:],
                                                        rs_h[h][:, it:it + 1])
                            # SWDGE casts bf16 -> f32 during the HBM write
                            nc.gpsimd.dma_start(
                                attn_o[h, it * P:(it + 1) * P, :], st["erow"][:])

                def normalize_pair(hp, ops_pair):
                    h0, h1 = 2 * hp, 2 * hp + 1
                    bps = psAB.tile([P, 2 * D], F32, name="psAB", tag="psAB")
                    for idx, h in enumerate((h0, h1)):
                        rs_row = bc_pool.tile([1, IB], F32, name="rs_row", tag="rs_row")
                        for it in range(NIT):
                            tps = psO.tile([P, D], F32, name="psO", tag="psO")
                            nc.tensor.transpose(tps[0:1, 0:P], rs_h[h][:, it:it + 1],
                                                ident[:])
                            nc.vector.tensor_copy(rs_row[:, it * P:(it + 1) * P],
                                                  tps[0:1, 0:P])
                        nc.tensor.matmul(bps[idx * DH:(idx + 1) * DH, 0:IB],
                                         ones_f32[:], rs_row[:],
                                         start=True, stop=True,
                                         tile_position=(0, idx * DH))
                    bsb = bc_pool.tile([P, IB], F32, name="bsb", tag="bsb")
                    nc.vector.tensor_copy(bsb[:], bps[:, 0:IB])
                    nc.vector.tensor_mul(ot_norm[hp][:], ops_pair[:], bsb[:])

                # ---------- stage 1 core + woven pair 0 ----------
                for jt in range(NJT):
                    emit_v(jt)
                for jc2 in range(JC2):
                    emit_kt_chunk(0, jc2)
                emit_qt(0)

                # weave remaining stage-1 work + all of pair 0 (A and B)
                stage1_rest = [("kt", ft, jc2) for ft in range(1, KC)
                               for jc2 in range(JC2)]
                stage1_rest += [("qt", ft, 0) for ft in range(1, KC)]
                s1_iter = iter(stage1_rest)
                open_b_state(0)
                b0_units = [(it, jc2) for it in range(NIT) for jc2 in range(JC2)]
                b0_iter = iter(b0_units)
                ops_pair0 = psO.tile([P, IB], F32, name="psO", tag="psO")
                for jt in range(NJT):
                    emit_a_step(0, jt, ops_pair0)
                    if jt % 2 == 0:
                        nxt = next(s1_iter, None)
                        if nxt is not None:
                            kind, ft, jc2 = nxt
                            if kind == "kt":
                                emit_kt_chunk(ft, jc2)
                            else:
                                emit_qt(ft)
                    elif jt >= 8:
                        u = next(b0_iter, None)
                        if u is not None:
                            emit_b_unit(0, *u)
                for nxt in s1_iter:
                    kind, ft, jc2 = nxt
                    if kind == "kt":
                        emit_kt_chunk(ft, jc2)
                    else:
                        emit_qt(ft)
                for u in b0_iter:
                    emit_b_unit(0, *u)
                normalize_pair(0, ops_pair0)

                # ---------- remaining pairs ----------
                for hp in range(1, HP):
                    ops_pair = psO.tile([P, IB], F32, name="psO", tag="psO")
                    for jt in range(NJT):
                        emit_a_step(hp, jt, ops_pair)
                    open_b_state(hp)
                    for it in range(NIT):
                        for jc2 in range(JC2):
                            emit_b_unit(hp, it, jc2)
                    normalize_pair(hp, ops_pair)

                # ---------- output projection ----------
                for it in range(NIT):
                    pps = psAB.tile([P, 2 * D], F32, name="psAB", tag="psAB")
                    for t in range(HP):
                        nc.tensor.matmul(pps[:, 0:D],
                                         ot_norm[t][:, it * P:(it + 1) * P],
                                         wout_sb[t][:], start=(t == 0), stop=False)
                    nc.tensor.matmul(pps[:, 0:D], ones_bf[:], bout_sb[:],
                                     start=False, stop=True)
                    osb = out_pool.tile([P, D], F32, name="osb", tag="osb")
                    nc.vector.tensor_copy(osb[:], pps[:, 0:D])
                    nc.sync.dma_start(out_o[it * P:(it + 1) * P, :], osb[:])

    nc.compile()
    return nc


def _get_nc():
    if "nc" not in _nc_cache:
        _nc_cache["nc"] = build_nc()
    return _nc_cache["nc"]


def _prep_in_maps(x, W_qkv, W_out, b_out):
    bf = ml_dtypes.bfloat16
    xT = np.ascontiguousarray(np.asarray(x, np.float32)[0].T).astype(bf)  # [512, 4096]
    wqkv = np.ascontiguousarray(np.asarray(W_qkv, np.float32)).astype(bf)
    wout = np.ascontiguousarray(np.asarray(W_out, np.float32)).astype(bf)
    bo = np.asarray(b_out, np.float32).reshape(1, D).astype(bf)
    in_maps = []
    for c in range(NC):
        xTq = np.ascontiguousarray(xT[:, c * IB:(c + 1) * IB])
        in_maps.append({"xT": xT, "xTq": xTq, "w_qkv": wqkv,
                        "w_out": wout, "b_out": bo})
    return in_maps


def run(inputs, trace=False, tmpdir=None):
    nc = _get_nc()
    in_maps = _prep_in_maps(inputs["x"], inputs["W_qkv"], inputs["W_out"],
                            inputs["b_out"])
    res = run_bass_kernel_spmd(nc, in_maps, core_ids=list(range(NC)),
                               trace=trace, tmpdir=tmpdir)
    outs = [res.results[c]["out"] for c in range(NC)]
    attns = [res.results[c]["attn"] for c in range(NC)]
    out_full = np.concatenate(outs, axis=0)[None]                   # [1, 4096, 512]
    attn_full = np.concatenate(attns, axis=1)[None]                 # [1, 8, 4096, 4096]
    return (out_full.astype(np.float32), attn_full.astype(np.float32)), res


def kernel(**inputs):
    (out_full, attn_full), _ = run(inputs, trace=False)
    return out_full, attn_full


# revision 23
# speedup vs baseline: 1.0573x; 1.0127x over previous
"""Trainium2 Bass kernel for nn_Attention (dense transformer attention).

Full module: qkv projection -> per-head softmax(q k^T / sqrt(d)) -> attn @ v
-> output projection (+bias). Returns (out, attn) like the reference.

Distribution: sequence-parallel over 8 NeuronCores. Each core owns a block of
512 query rows and computes, fully on-device:
  - k^T and v for ALL 4096 tokens (replicated compute; beats an all-gather,
    whose fixed cost measures ~200us on this stack),
  - q^T for its own 512 rows,
  - per head pair: dots^T -> exp -> E^T (bf16) feeding O^T = (E @ v)^T on the
    TensorEngine (two heads packed into PE row groups / PSUM col groups),
    then row-major dots -> exp (with row-sum accumulation) -> normalized
    attention rows written straight to HBM,
  - O^T normalized via PE-transposed reciprocal row sums, then the output
    projection for its own rows (bias folded in as a K=1 matmul).

Pair 0's phase A AND phase B are woven into the tail of stage 1 so the
ScalarEngine and DMA engines are busy while the TensorEngine grinds through
the qkv projections (keeps the PE HAM clock-gate warm, overlaps the first
16MB of attention writes).

Host side only reshapes/casts inputs (x^T, weights to bf16) and concatenates
the per-core outputs. No FLOPs on the host.
"""

import sys

sys.path.insert(0, "/opt/trn_rl_repo")

import numpy as np
import ml_dtypes

import concourse.bass as bass
import concourse.mybir as mybir
import concourse.tile as tile
from concourse import bacc
from concourse.bass_utils import run_bass_kernel_spmd
from concourse.masks import make_identity

F32 = mybir.dt.float32
BF16 = mybir.dt.bfloat16
EXP = mybir.ActivationFunctionType.Exp
AX_X = mybir.AxisListType.X

N = 4096          # sequence length
D = 512           # model dim
H = 8             # heads
DH = 64           # head dim
NC = 8            # cores
IB = N // NC      # query rows per core = 512
P = 128
SCALE = DH ** -0.5

KC = D // P        # 4 contraction chunks for the qkv projections
NJT = N // P       # 32 key tiles
NIT = IB // P      # 4 query row-tiles per core
HP = H // 2        # head pairs
JC2 = N // (2 * D) # 4 row-major 1024-wide chunks

_nc_cache = {}


def build_nc():
    """Build the single-core Bass program (same program runs SPMD on 8 cores)."""
    nc = bacc.Bacc("TRN2", target_bir_lowering=False, debug=False,
                   enable_asserts=False)

    xT = nc.declare_dram_parameter("xT", [D, N], BF16, isOutput=False)
    xTq = nc.declare_dram_parameter("xTq", [D, IB], BF16, isOutput=False)
    wqkv = nc.declare_dram_parameter("w_qkv", [D, 3 * D], BF16, isOutput=False)
    wout = nc.declare_dram_parameter("w_out", [D, D], BF16, isOutput=False)
    bout = nc.declare_dram_parameter("b_out", [1, D], BF16, isOutput=False)
    attn_o = nc.declare_dram_parameter("attn", [H, IB, N], F32, isOutput=True)
    out_o = nc.declare_dram_parameter("out", [IB, D], F32, isOutput=True)

    with tile.TileContext(nc) as tc:
        with (
            tc.tile_pool(name="consts", bufs=1) as consts,
            tc.tile_pool(name="kt2", bufs=1) as kt2_pool,
            tc.tile_pool(name="vsb", bufs=1) as v_pool,
            tc.tile_pool(name="qt2", bufs=1) as qt2_pool,
            tc.tile_pool(name="otp", bufs=1) as ot_pool,
            tc.tile_pool(name="srows", bufs=1) as srow_pool,
            tc.tile_pool(name="et", bufs=4) as et_pool,
            tc.tile_pool(name="erow", bufs=8) as erow_pool,
            tc.tile_pool(name="sacc", bufs=4) as sacc_pool,
            tc.tile_pool(name="bcast", bufs=1) as bc_pool,
            tc.tile_pool(name="osb", bufs=1) as out_pool,
            tc.tile_pool(name="psAB", bufs=3, space="PSUM") as psAB,
            tc.tile_pool(name="psO", bufs=2, space="PSUM") as psO,
        ):
            wout_sb = [consts.tile([P, D], BF16, name=f"wout{t}", tag=f"wout{t}")
                       for t in range(KC)]
            bout_sb = consts.tile([1, D], BF16, name="bout", tag="bout")
            ones_bf = consts.tile([1, P], BF16, name="ones_bf", tag="ones_bf")
            ones_f32 = consts.tile([1, DH], F32, name="ones_f32", tag="ones_f32")
            ident = consts.tile([P, P], F32, name="ident", tag="ident")
            for t in range(KC):
                nc.sync.dma_start(wout_sb[t][:], wout[t * P:(t + 1) * P, :])
            nc.sync.dma_start(bout_sb[:], bout[:])
            nc.vector.memset(ones_bf[:], 1.0)
            nc.vector.memset(ones_f32[:], 1.0)
            make_identity(nc, ident[:])

            kt2 = [kt2_pool.tile([P, N], BF16, name=f"kt2_{hp}", tag=f"kt2_{hp}")
                   for hp in range(HP)]
            v_sb = [v_pool.tile([P, D], BF16, name=f"v{jt}", tag=f"v{jt}")
                    for jt in range(NJT)]
            qt2 = [qt2_pool.tile([P, IB], BF16, name=f"qt2_{hp}", tag=f"qt2_{hp}")
                   for hp in range(HP)]
            ot_norm = [ot_pool.tile([P, IB], BF16, name=f"otn{t}", tag=f"otn{t}")
                       for t in range(HP)]
            rs_h = [srow_pool.tile([P, NIT], F32, name=f"rs_{h}", tag=f"rs_{h}")
                    for h in range(H)]

            # ================= stage 1 emitters =================
            with tc.tile_pool(name="xin", bufs=1) as xin_pool:
                xT_sb = [xin_pool.tile([P, N], BF16, name=f"xT{kc}", tag=f"xT{kc}")
                         for kc in range(KC)]
                xTq_sb = [xin_pool.tile([P, IB], BF16, name=f"xTq{kc}", tag=f"xTq{kc}")
                          for kc in range(KC)]
                wqkv_sb = [xin_pool.tile([P, 3 * D], BF16, name=f"wqkv{kc}", tag=f"wqkv{kc}")
                           for kc in range(KC)]
                for kc in range(KC):
                    nc.sync.dma_start(xT_sb[kc][:], xT[kc * P:(kc + 1) * P, :])
                    nc.sync.dma_start(xTq_sb[kc][:], xTq[kc * P:(kc + 1) * P, :])
                    nc.sync.dma_start(wqkv_sb[kc][:], wqkv[kc * P:(kc + 1) * P, :])

                def emit_kt_chunk(ft, jc2):
                    ps = psAB.tile([P, 2 * D], F32, name="psAB", tag="psAB")
                    for half in range(2):
                        jc = 2 * jc2 + half
                        for kc in range(KC):
                            nc.tensor.matmul(
                                ps[:, half * D:(half + 1) * D],
                                wqkv_sb[kc][:, D + ft * P: D + (ft + 1) * P],
                                xT_sb[kc][:, jc * D:(jc + 1) * D],
                                start=(kc == 0), stop=(kc == KC - 1))
                    nc.vector.tensor_copy(
                        kt2[ft][:, jc2 * 2 * D:(jc2 + 1) * 2 * D], ps[:])

                def emit_qt(ft):
                    ps = psAB.tile([P, 2 * D], F32, name="psAB", tag="psAB")
                    for kc in range(KC):
                        nc.tensor.matmul(
                            ps[:, 0:IB],
                            wqkv_sb[kc][:, ft * P:(ft + 1) * P],
                            xTq_sb[kc][:],
                            start=(kc == 0), stop=(kc == KC - 1))
                    nc.vector.tensor_copy(qt2[ft][:], ps[:, 0:IB])

                def emit_v(jt):
                    ps = psO.tile([P, D], F32, name="psO", tag="psO")
                    for kc in range(KC):
                        nc.tensor.matmul(
                            ps[:], xT_sb[kc][:, jt * P:(jt + 1) * P],
                            wqkv_sb[kc][:, 2 * D:3 * D],
                            start=(kc == 0), stop=(kc == KC - 1))
                    nc.vector.tensor_copy(v_sb[jt][:], ps[:])

                # ================= stage 2 emitters =================
                def emit_a_step(hp, jt, ops_pair):
                    h0, h1 = 2 * hp, 2 * hp + 1
                    dps = psAB.tile([P, 2 * IB], F32, name="psAB", tag="psAB")
                    nc.tensor.matmul(
                        dps[:, 0:IB], kt2[hp][0:DH, jt * P:(jt + 1) * P],
                        qt2[hp][0:DH, :], start=True, stop=True)
                    nc.tensor.matmul(
                        dps[:, IB:2 * IB], kt2[hp][DH:P, jt * P:(jt + 1) * P],
                        qt2[hp][DH:P, :], start=True, stop=True)
                    et = et_pool.tile([P, 2 * IB], BF16, name="et", tag="et")
                    nc.scalar.activation(et[:], dps[:], EXP, scale=SCALE)
                    nc.tensor.matmul(ops_pair[0:DH, :],
                                     v_sb[jt][:, h0 * DH:(h0 + 1) * DH],
                                     et[:, 0:IB],
                                     start=(jt == 0), stop=(jt == NJT - 1))
                    nc.tensor.matmul(ops_pair[DH:P, :],
                                     v_sb[jt][:, h1 * DH:(h1 + 1) * DH],
                                     et[:, IB:2 * IB],
                                     start=(jt == 0), stop=(jt == NJT - 1),
                                     tile_position=(0, DH))

                b_state = {}

                def open_b_state(hp):
                    for h in (2 * hp, 2 * hp + 1):
                        for it in range(NIT):
                            b_state[(h, it)] = {
                                "erow": erow_pool.tile([P, N], BF16, name="erow", tag="erow"),
                                "sacc": sacc_pool.tile([P, JC2], F32, name="sacc", tag="sacc"),
                            }

                def emit_b_unit(hp, it, jc2):
                    h0, h1 = 2 * hp, 2 * hp + 1
                    rps0 = psAB.tile([P, 2 * D], F32, name="psAB", tag="psAB")
                    rps1 = psAB.tile([P, 2 * D], F32, name="psAB", tag="psAB")
                    for half in range(2):
                        jcs = (2 * jc2 + half) * D
                        for rps, pb in ((rps0, 0), (rps1, DH)):
                            nc.tensor.matmul(
                                rps[:, half * D:(half + 1) * D],
                                qt2[hp][pb:pb + DH, it * P:(it + 1) * P],
                                kt2[hp][pb:pb + DH, jcs:jcs + D],
                                start=True, stop=True)
                    for rps, h in ((rps0, h0), (rps1, h1)):
                        st = b_state[(h, it)]
                        nc.scalar.activation(
                            st["erow"][:, jc2 * 2 * D:(jc2 + 1) * 2 * D], rps[:],
                            EXP, scale=SCALE,
                            accum_out=st["sacc"][:, jc2:jc2 + 1])
                        if jc2 == JC2 - 1:
                            stot = sacc_pool.tile([P, 1], F32, name="stot", tag="stot")
                            nc.vector.reduce_sum(stot[:], st["sacc"][:], axis=AX_X)
                            nc.vector.reciprocal(rs_h[h][:, it:it + 1], stot[:])
                            nc.vector.tensor_scalar_mul(st["erow"][:], st["erow"][:],
                                                        rs_h[h][:, it:it + 1])
                            # SWDGE casts bf16 -> f32 during the HBM write
                            nc.gpsimd.dma_start(
                                attn_o[h, it * P:(it + 1) * P, :], st["erow"][:])

                def normalize_pair(hp, ops_pair):
                    h0, h1 = 2 * hp, 2 * hp + 1
                    bps = psAB.tile([P, 2 * D], F32, name="psAB", tag="psAB")
                    for idx, h in enumerate((h0, h1)):
                        rs_row = bc_pool.tile([1, IB], F32, name="rs_row", tag="rs_row")
                        for it in range(NIT):
                            tps = psO.tile([P, D], F32, name="psO", tag="psO")
                            nc.tensor.transpose(tps[0:1, 0:P], rs_h[h][:, it:it + 1],
                                                ident[:])
                            nc.vector.tensor_copy(rs_row[:, it * P:(it + 1) * P],
                                                  tps[0:1, 0:P])
                        nc.tensor.matmul(bps[idx * DH:(idx + 1) * DH, 0:IB],
                                         ones_f32[:], rs_row[:],
                                         start=True, stop=True,
                                         tile_position=(0, idx * DH))
                    bsb = bc_pool.tile([P, IB], F32, name="bsb", tag="bsb")
                    nc.vector.tensor_copy(bsb[:], bps[:, 0:IB])
                    nc.vector.tensor_mul(ot_norm[hp][:], ops_pair[:], bsb[:])

                # ---------- stage 1 core + woven pair 0 ----------
                # k/q for pair 0 first so the first exp starts ASAP; v tiles
                # are produced just-in-time inside the phase-A weave.
                for jc2 in range(JC2):
                    emit_kt_chunk(0, jc2)
                emit_qt(0)

                stage1_rest = [("kt", ft, jc2) for ft in range(1, KC)
                               for jc2 in range(JC2)]
                stage1_rest += [("qt", ft, 0) for ft in range(1, KC)]
                s1_iter = iter(stage1_rest)
                open_b_state(0)
                b0_units = [(it, jc2) for it in range(NIT) for jc2 in range(JC2)]
                b0_iter = iter(b0_units)
                ops_pair0 = psO.tile([P, IB], F32, name="psO", tag="psO")
                for jt in range(NJT):
                    emit_v(jt)
                    emit_a_step(0, jt, ops_pair0)
                    if jt % 2 == 0:
                        nxt = next(s1_iter, None)
                        if nxt is not None:
                            kind, ft, jc2 = nxt
                            if kind == "kt":
                                emit_kt_chunk(ft, jc2)
                            else:
                                emit_qt(ft)
                    elif jt >= 5:
                        u = next(b0_iter, None)
                        if u is not None:
                            emit_b_unit(0, *u)
                for nxt in s1_iter:
                    kind, ft, jc2 = nxt
                    if kind == "kt":
                        emit_kt_chunk(ft, jc2)
                    else:
                        emit_qt(ft)
                for u in b0_iter:
                    emit_b_unit(0, *u)
                normalize_pair(0, ops_pair0)

                # ---------- remaining pairs ----------
                for hp in range(1, HP):
                    ops_pair = psO.tile([P, IB], F32, name="psO", tag="psO")
                    for jt in range(NJT):
                        emit_a_step(hp, jt, ops_pair)
                    open_b_state(hp)
                    for it in range(NIT):
                        for jc2 in range(JC2):
                            emit_b_unit(hp, it, jc2)
                    normalize_pair(hp, ops_pair)

                # ---------- output projection ----------
                for it in range(NIT):
                    pps = psAB.tile([P, 2 * D], F32, name="psAB", tag="psAB")
                    for t in range(HP):
                        nc.tensor.matmul(pps[:, 0:D],
                                         ot_norm[t][:, it * P:(it + 1) * P],
                                         wout_sb[t][:], start=(t == 0), stop=False)
                    nc.tensor.matmul(pps[:, 0:D], ones_bf[:], bout_sb[:],
                                     start=False, stop=True)
                    osb = out_pool.tile([P, D], F32, name="osb", tag="osb")
                    nc.vector.tensor_copy(osb[:], pps[:, 0:D])
                    nc.sync.dma_start(out_o[it * P:(it + 1) * P, :], osb[:])

    nc.compile()
    return nc


def _get_nc():
    if "nc" not in _nc_cache:
        _nc_cache["nc"] = build_nc()
    return _nc_cache["nc"]


def _prep_in_maps(x, W_qkv, W_out, b_out):
    bf = ml_dtypes.bfloat16
    xT = np.ascontiguousarray(np.asarray(x, np.float32)[0].T).astype(bf)  # [512, 4096]
    wqkv = np.ascontiguousarray(np.asarray(W_qkv, np.float32)).astype(bf)
    wout = np.ascontiguousarray(np.asarray(W_out, np.float32)).astype(bf)
    bo = np.asarray(b_out, np.float32).reshape(1, D).astype(bf)
    in_maps = []
    for c in range(NC):
        xTq = np.ascontiguousarray(xT[:, c * IB:(c + 1) * IB])
        in_maps.append({"xT": xT, "xTq": xTq, "w_qkv": wqkv,
                        "w_out": wout, "b_out": bo})
    return in_maps


def run(inputs, trace=False, tmpdir=None):
    nc = _get_nc()
    in_maps = _prep_in_maps(inputs["x"], inputs["W_qkv"], inputs["W_out"],
                            inputs["b_out"])
    res = run_bass_kernel_spmd(nc, in_maps, core_ids=list(range(NC)),
                               trace=trace, tmpdir=tmpdir)
    outs = [res.results[c]["out"] for c in range(NC)]
    attns = [res.results[c]["attn"] for c in range(NC)]
    out_full = np.concatenate(outs, axis=0)[None]                   # [1, 4096, 512]
    attn_full = np.concatenate(attns, axis=1)[None]                 # [1, 8, 4096, 4096]
    return (out_full.astype(np.float32), attn_full.astype(np.float32)), res


def kernel(**inputs):
    (out_full, attn_full), _ = run(inputs, trace=False)
    return out_full, attn_full


# revision 27
# speedup vs baseline: 1.0733x; 1.0151x over previous
"""Trainium2 Bass kernel for nn_Attention (dense transformer attention).

Full module: qkv projection -> per-head softmax(q k^T / sqrt(d)) -> attn @ v
-> output projection (+bias). Returns (out, attn) like the reference.

Distribution: sequence-parallel over 8 NeuronCores. Each core owns a block of
512 query rows and computes, fully on-device:
  - k^T and v for ALL 4096 tokens (replicated compute; beats an all-gather,
    whose fixed cost measures ~200us on this stack),
  - q^T for its own 512 rows,
  - per head pair: dots^T -> exp -> E^T (bf16) feeding O^T = (E @ v)^T on the
    TensorEngine (two heads packed into PE row groups / PSUM col groups),
    then row-major dots -> exp (with row-sum accumulation) -> normalized
    attention rows written straight to HBM,
  - O^T normalized via PE-transposed reciprocal row sums, then the output
    projection for its own rows (bias folded in as a K=1 matmul).

Pair 0's phase A AND phase B are woven into the tail of stage 1 so the
ScalarEngine and DMA engines are busy while the TensorEngine grinds through
the qkv projections (keeps the PE HAM clock-gate warm, overlaps the first
16MB of attention writes).

Host side only reshapes/casts inputs (x^T, weights to bf16) and concatenates
the per-core outputs. No FLOPs on the host.
"""

import sys

sys.path.insert(0, "/opt/trn_rl_repo")

import numpy as np
import ml_dtypes

import concourse.bass as bass
import concourse.mybir as mybir
import concourse.tile as tile
from concourse import bacc
from concourse.bass_utils import run_bass_kernel_spmd
from concourse.masks import make_identity

F32 = mybir.dt.float32
BF16 = mybir.dt.bfloat16
EXP = mybir.ActivationFunctionType.Exp
AX_X = mybir.AxisListType.X

N = 4096          # sequence length
D = 512           # model dim
H = 8             # heads
DH = 64           # head dim
NC = 8            # cores
IB = N // NC      # query rows per core = 512
P = 128
SCALE = DH ** -0.5

KC = D // P        # 4 contraction chunks for the qkv projections
NJT = N // P       # 32 key tiles
NIT = IB // P      # 4 query row-tiles per core
HP = H // 2        # head pairs
JC2 = N // (2 * D) # 4 row-major 1024-wide chunks

_nc_cache = {}


def build_nc():
    """Build the single-core Bass program (same program runs SPMD on 8 cores)."""
    nc = bacc.Bacc("TRN2", target_bir_lowering=False, debug=False,
                   enable_asserts=False)

    xT = nc.declare_dram_parameter("xT", [D, N], BF16, isOutput=False)
    xTq = nc.declare_dram_parameter("xTq", [D, IB], BF16, isOutput=False)
    wqkv = nc.declare_dram_parameter("w_qkv", [D, 3 * D], BF16, isOutput=False)
    wout = nc.declare_dram_parameter("w_out", [D, D], BF16, isOutput=False)
    bout = nc.declare_dram_parameter("b_out", [1, D], BF16, isOutput=False)
    attn_o = nc.declare_dram_parameter("attn", [H, IB, N], F32, isOutput=True)
    out_o = nc.declare_dram_parameter("out", [IB, D], F32, isOutput=True)

    with tile.TileContext(nc) as tc:
        with (
            tc.tile_pool(name="consts", bufs=1) as consts,
            tc.tile_pool(name="kt2", bufs=1) as kt2_pool,
            tc.tile_pool(name="vsb", bufs=1) as v_pool,
            tc.tile_pool(name="qt2", bufs=1) as qt2_pool,
            tc.tile_pool(name="otp", bufs=1) as ot_pool,
            tc.tile_pool(name="srows", bufs=1) as srow_pool,
            tc.tile_pool(name="et", bufs=5) as et_pool,
            tc.tile_pool(name="erow", bufs=8) as erow_pool,
            tc.tile_pool(name="sacc", bufs=4) as sacc_pool,
            tc.tile_pool(name="bcast", bufs=1) as bc_pool,
            tc.tile_pool(name="osb", bufs=1) as out_pool,
            tc.tile_pool(name="psAB", bufs=3, space="PSUM") as psAB,
            tc.tile_pool(name="psO", bufs=2, space="PSUM") as psO,
        ):
            wout_sb = [consts.tile([P, D], BF16, name=f"wout{t}", tag=f"wout{t}")
                       for t in range(KC)]
            bout_sb = consts.tile([1, D], BF16, name="bout", tag="bout")
            ones_bf = consts.tile([1, P], BF16, name="ones_bf", tag="ones_bf")
            ones_f32 = consts.tile([1, DH], F32, name="ones_f32", tag="ones_f32")
            ident = consts.tile([P, P], F32, name="ident", tag="ident")
            nc.vector.memset(ones_bf[:], 1.0)
            nc.vector.memset(ones_f32[:], 1.0)
            make_identity(nc, ident[:])

            kt2 = [kt2_pool.tile([P, N], BF16, name=f"kt2_{hp}", tag=f"kt2_{hp}")
                   for hp in range(HP)]
            v_sb = [v_pool.tile([P, D], BF16, name=f"v{jt}", tag=f"v{jt}")
                    for jt in range(NJT)]
            qt2 = [qt2_pool.tile([P, IB], BF16, name=f"qt2_{hp}", tag=f"qt2_{hp}")
                   for hp in range(HP)]
            ot_norm = [ot_pool.tile([P, IB], BF16, name=f"otn{t}", tag=f"otn{t}")
                       for t in range(HP)]
            rs_h = [srow_pool.tile([P, NIT], F32, name=f"rs_{h}", tag=f"rs_{h}")
                    for h in range(H)]

            # ================= stage 1 emitters =================
            with tc.tile_pool(name="xin", bufs=1) as xin_pool:
                xT_sb = [xin_pool.tile([P, N], BF16, name=f"xT{kc}", tag=f"xT{kc}")
                         for kc in range(KC)]
                xTq_sb = [xin_pool.tile([P, IB], BF16, name=f"xTq{kc}", tag=f"xTq{kc}")
                          for kc in range(KC)]
                wqkv_sb = [xin_pool.tile([P, 3 * D], BF16, name=f"wqkv{kc}", tag=f"wqkv{kc}")
                           for kc in range(KC)]
                # quarter the xT loads (column-wise) so the first k^T chunk's
                # inputs land as early as possible
                for q in range(4):
                    for kc in range(KC):
                        if q == 0:
                            nc.sync.dma_start(wqkv_sb[kc][:],
                                              wqkv[kc * P:(kc + 1) * P, :])
                        nc.sync.dma_start(
                            xT_sb[kc][:, q * (N // 4):(q + 1) * (N // 4)],
                            xT[kc * P:(kc + 1) * P, q * (N // 4):(q + 1) * (N // 4)])
                        if q == 1:
                            nc.sync.dma_start(xTq_sb[kc][:],
                                              xTq[kc * P:(kc + 1) * P, :])
                for t in range(KC):
                    nc.sync.dma_start(wout_sb[t][:], wout[t * P:(t + 1) * P, :])
                nc.sync.dma_start(bout_sb[:], bout[:])

                def emit_kt_chunk(ft, jc2):
                    ps = psAB.tile([P, 2 * D], F32, name="psAB", tag="psAB")
                    for half in range(2):
                        jc = 2 * jc2 + half
                        for kc in range(KC):
                            nc.tensor.matmul(
                                ps[:, half * D:(half + 1) * D],
                                wqkv_sb[kc][:, D + ft * P: D + (ft + 1) * P],
                                xT_sb[kc][:, jc * D:(jc + 1) * D],
                                start=(kc == 0), stop=(kc == KC - 1))
                    nc.vector.tensor_copy(
                        kt2[ft][:, jc2 * 2 * D:(jc2 + 1) * 2 * D], ps[:])

                def emit_qt(ft):
                    ps = psAB.tile([P, 2 * D], F32, name="psAB", tag="psAB")
                    for kc in range(KC):
                        nc.tensor.matmul(
                            ps[:, 0:IB],
                            wqkv_sb[kc][:, ft * P:(ft + 1) * P],
                            xTq_sb[kc][:],
                            start=(kc == 0), stop=(kc == KC - 1))
                    nc.vector.tensor_copy(qt2[ft][:], ps[:, 0:IB])

                def emit_v(jt):
                    ps = psO.tile([P, D], F32, name="psO", tag="psO")
                    for kc in range(KC):
                        nc.tensor.matmul(
                            ps[:], xT_sb[kc][:, jt * P:(jt + 1) * P],
                            wqkv_sb[kc][:, 2 * D:3 * D],
                            start=(kc == 0), stop=(kc == KC - 1))
                    nc.vector.tensor_copy(v_sb[jt][:], ps[:])

                # ================= stage 2 emitters =================
                def emit_a_step(hp, jt, ops_pair):
                    h0, h1 = 2 * hp, 2 * hp + 1
                    dps = psAB.tile([P, 2 * IB], F32, name="psAB", tag="psAB")
                    nc.tensor.matmul(
                        dps[:, 0:IB], kt2[hp][0:DH, jt * P:(jt + 1) * P],
                        qt2[hp][0:DH, :], start=True, stop=True)
                    nc.tensor.matmul(
                        dps[:, IB:2 * IB], kt2[hp][DH:P, jt * P:(jt + 1) * P],
                        qt2[hp][DH:P, :], start=True, stop=True)
                    et = et_pool.tile([P, 2 * IB], BF16, name="et", tag="et")
                    nc.scalar.activation(et[:], dps[:], EXP, scale=SCALE)
                    nc.tensor.matmul(ops_pair[0:DH, :],
                                     v_sb[jt][:, h0 * DH:(h0 + 1) * DH],
                                     et[:, 0:IB],
                                     start=(jt == 0), stop=(jt == NJT - 1))
                    nc.tensor.matmul(ops_pair[DH:P, :],
                                     v_sb[jt][:, h1 * DH:(h1 + 1) * DH],
                                     et[:, IB:2 * IB],
                                     start=(jt == 0), stop=(jt == NJT - 1),
                                     tile_position=(0, DH))

                b_state = {}

                def open_b_state(hp):
                    for h in (2 * hp, 2 * hp + 1):
                        for it in range(NIT):
                            b_state[(h, it)] = {
                                "erow": erow_pool.tile([P, N], BF16, name="erow", tag="erow"),
                                "sacc": sacc_pool.tile([P, JC2], F32, name="sacc", tag="sacc"),
                            }

                def emit_b_unit(hp, it, jc2):
                    h0, h1 = 2 * hp, 2 * hp + 1
                    rps0 = psAB.tile([P, 2 * D], F32, name="psAB", tag="psAB")
                    rps1 = psAB.tile([P, 2 * D], F32, name="psAB", tag="psAB")
                    for half in range(2):
                        jcs = (2 * jc2 + half) * D
                        for rps, pb in ((rps0, 0), (rps1, DH)):
                            nc.tensor.matmul(
                                rps[:, half * D:(half + 1) * D],
                                qt2[hp][pb:pb + DH, it * P:(it + 1) * P],
                                kt2[hp][pb:pb + DH, jcs:jcs + D],
                                start=True, stop=True)
                    for rps, h in ((rps0, h0), (rps1, h1)):
                        st = b_state[(h, it)]
                        nc.scalar.activation(
                            st["erow"][:, jc2 * 2 * D:(jc2 + 1) * 2 * D], rps[:],
                            EXP, scale=SCALE,
                            accum_out=st["sacc"][:, jc2:jc2 + 1])
                        if jc2 == JC2 - 1:
                            stot = sacc_pool.tile([P, 1], F32, name="stot", tag="stot")
                            nc.vector.reduce_sum(stot[:], st["sacc"][:], axis=AX_X)
                            nc.vector.reciprocal(rs_h[h][:, it:it + 1], stot[:])
                            nc.vector.tensor_scalar_mul(st["erow"][:], st["erow"][:],
                                                        rs_h[h][:, it:it + 1])
                            # SWDGE casts bf16 -> f32 during the HBM write
                            nc.gpsimd.dma_start(
                                attn_o[h, it * P:(it + 1) * P, :], st["erow"][:])

                def normalize_pair(hp, ops_pair):
                    h0, h1 = 2 * hp, 2 * hp + 1
                    bps = psAB.tile([P, 2 * D], F32, name="psAB", tag="psAB")
                    for idx, h in enumerate((h0, h1)):
                        rs_row = bc_pool.tile([1, IB], F32, name="rs_row", tag="rs_row")
                        for it in range(NIT):
                            tps = psO.tile([P, D], F32, name="psO", tag="psO")
                            nc.tensor.transpose(tps[0:1, 0:P], rs_h[h][:, it:it + 1],
                                                ident[:])
                            nc.vector.tensor_copy(rs_row[:, it * P:(it + 1) * P],
                                                  tps[0:1, 0:P])
                        nc.tensor.matmul(bps[idx * DH:(idx + 1) * DH, 0:IB],
                                         ones_f32[:], rs_row[:],
                                         start=True, stop=True,
                                         tile_position=(0, idx * DH))
                    bsb = bc_pool.tile([P, IB], F32, name="bsb", tag="bsb")
                    nc.vector.tensor_copy(bsb[:], bps[:, 0:IB])
                    nc.vector.tensor_mul(ot_norm[hp][:], ops_pair[:], bsb[:])

                # ---------- stage 1 core + woven pair 0 ----------
                # k/q for pair 0 first so the first exp starts ASAP; v tiles
                # are produced just-in-time inside the phase-A weave.
                for jc2 in range(JC2):
                    emit_kt_chunk(0, jc2)
                emit_qt(0)

                stage1_rest = [("kt", ft, jc2) for ft in range(1, KC)
                               for jc2 in range(JC2)]
                stage1_rest += [("qt", ft, 0) for ft in range(1, KC)]
                s1_iter = iter(stage1_rest)
                open_b_state(0)
                b0_units = [(it, jc2) for it in range(NIT) for jc2 in range(JC2)]
                b0_iter = iter(b0_units)
                ops_pair0 = psO.tile([P, IB], F32, name="psO", tag="psO")
                for jt in range(NJT):
                    emit_v(jt)
                    emit_a_step(0, jt, ops_pair0)
                    if jt % 2 == 0:
                        nxt = next(s1_iter, None)
                        if nxt is not None:
                            kind, ft, jc2 = nxt
                            if kind == "kt":
                                emit_kt_chunk(ft, jc2)
                            else:
                                emit_qt(ft)
                    elif jt >= 5:
                        u = next(b0_iter, None)
                        if u is not None:
                            emit_b_unit(0, *u)
                for nxt in s1_iter:
                    kind, ft, jc2 = nxt
                    if kind == "kt":
                        emit_kt_chunk(ft, jc2)
                    else:
                        emit_qt(ft)
                for u in b0_iter:
                    emit_b_unit(0, *u)
                normalize_pair(0, ops_pair0)

                # ---------- remaining pairs ----------
                for hp in range(1, HP):
                    ops_pair = psO.tile([P, IB], F32, name="psO", tag="psO")
                    for jt in range(NJT):
                        emit_a_step(hp, jt, ops_pair)
                    open_b_state(hp)
                    for it in range(NIT):
                        for jc2 in range(JC2):
                            emit_b_unit(hp, it, jc2)
                    normalize_pair(hp, ops_pair)

                # ---------- output projection ----------
                for it in range(NIT):
                    pps = psAB.tile([P, 2 * D], F32, name="psAB", tag="psAB")
                    for t in range(HP):
                        nc.tensor.matmul(pps[:, 0:D],
                                         ot_norm[t][:, it * P:(it + 1) * P],
                                         wout_sb[t][:], start=(t == 0), stop=False)
                    nc.tensor.matmul(pps[:, 0:D], ones_bf[:], bout_sb[:],
                                     start=False, stop=True)
                    osb = out_pool.tile([P, D], F32, name="osb", tag="osb")
                    nc.vector.tensor_copy(osb[:], pps[:, 0:D])
                    nc.sync.dma_start(out_o[it * P:(it + 1) * P, :], osb[:])

    nc.compile()
    return nc


def _get_nc():
    if "nc" not in _nc_cache:
        _nc_cache["nc"] = build_nc()
    return _nc_cache["nc"]


def _prep_in_maps(x, W_qkv, W_out, b_out):
    bf = ml_dtypes.bfloat16
    xT = np.ascontiguousarray(np.asarray(x, np.float32)[0].T).astype(bf)  # [512, 4096]
    wqkv = np.ascontiguousarray(np.asarray(W_qkv, np.float32)).astype(bf)
    wout = np.ascontiguousarray(np.asarray(W_out, np.float32)).astype(bf)
    bo = np.asarray(b_out, np.float32).reshape(1, D).astype(bf)
    in_maps = []
    for c in range(NC):
        xTq = np.ascontiguousarray(xT[:, c * IB:(c + 1) * IB])
        in_maps.append({"xT": xT, "xTq": xTq, "w_qkv": wqkv,
                        "w_out": wout, "b_out": bo})
    return in_maps


def run(inputs, trace=False, tmpdir=None):
    nc = _get_nc()
    in_maps = _prep_in_maps(inputs["x"], inputs["W_qkv"], inputs["W_out"],
                            inputs["b_out"])
    res = run_bass_kernel_spmd(nc, in_maps, core_ids=list(range(NC)),
                               trace=trace, tmpdir=tmpdir)
    outs = [res.results[c]["out"] for c in range(NC)]
    attns = [res.results[c]["attn"] for c in range(NC)]
    out_full = np.concatenate(outs, axis=0)[None]                   # [1, 4096, 512]
    attn_full = np.concatenate(attns, axis=1)[None]                 # [1, 8, 4096, 4096]
    return (out_full.astype(np.float32), attn_full.astype(np.float32)), res


def kernel(**inputs):
    (out_full, attn_full), _ = run(inputs, trace=False)
    return out_full, attn_full
